# revision 1
# baseline (speedup 1.0000x reference)
"""BFAN sparse-attention similarity kernel, sharded across 8 trn2 NeuronCores.

Strategy (per sharding hint): shard the caption axis across the 8 cores.
Each core computes a (n_image=128, n_caption/8=16) block of the similarity
matrix with images replicated; the host concatenates the column blocks.

Shapes (hardcoded per the problem spec):
  images:   (128, 36, 1024) f32
  captions: (128, 32, 1024) f32
  cap_lens: (128,) int32  (all == 32; unused by the math)
Output: (128, 128) f32  -> out[i, c]
"""

import numpy as np

N_IMAGE, N_REGIONS = 128, 36
N_CAPTION, N_WORD = 128, 32
D = 1024
LAMBDA_SOFTMAX = 20.0
EPS = 1e-8
LEAKY_SLOPE = 0.1
N_CORES = 8

_compiled = {}


def _get_pfn():
    if "pfn" in _compiled:
        return _compiled["pfn"]

    import jax
    import jax.numpy as jnp

    def _l2norm(x, axis):
        return x / (jnp.linalg.norm(x, axis=axis, keepdims=True) + EPS)

    def _focal_attn(raw):
        a = jax.nn.leaky_relu(raw, LEAKY_SLOPE)
        a = _l2norm(a, axis=-1)
        a = jnp.swapaxes(a, -1, -2)
        a = jax.nn.softmax(a * LAMBDA_SOFTMAX, axis=-1)
        S = a.shape[-1]
        funcF = a * S - jnp.sum(a, axis=-1, keepdims=True)
        fattn = jnp.where(funcF > 0, 1.0, 0.0).astype(a.dtype)
        tmp = fattn * a
        return tmp / jnp.sum(tmp, axis=-1, keepdims=True)

    def _shard_fn(images, captions):
        # images: (I, R, d); captions: (C_loc, W, d)
        # Use the Gram-matrix reformulation to avoid materializing the
        # (C, I, W, d) weighted-context tensors: for attention weights re and
        # raw similarity matrix raw,
        #   cos(query_q, wctx_q) = sum_s re[q,s] raw[s,q]
        #       / (||query_q|| * sqrt(re[q,:] @ G @ re[q,:]^T))
        # where G is the Gram matrix of the context set.
        raw = jnp.einsum("ird,cwd->cirw", images, captions)  # (C,I,R,W)
        g_img = jnp.einsum("ird,isd->irs", images, images)  # (I,R,R)
        g_cap = jnp.einsum("cwd,cvd->cwv", captions, captions)  # (C,W,W)
        img_norm = jnp.maximum(jnp.linalg.norm(images, axis=-1), EPS)  # (I,R)
        cap_norm = jnp.maximum(jnp.linalg.norm(captions, axis=-1), EPS)  # (C,W)

        # ---- t2i: query = caption words (W), context = image regions (R) ----
        re_t2i = _focal_attn(raw)  # (C,I,W,R)
        num = jnp.einsum("ciwr,cirw->ciw", re_t2i, raw)
        qn = jnp.einsum("ciwr,irs,ciws->ciw", re_t2i, g_img, re_t2i)
        qn = jnp.maximum(jnp.sqrt(qn), EPS)
        t2i = (num / (cap_norm[:, None, :] * qn)).mean(axis=-1)  # (C,I)

        # ---- i2t: query = image regions (R), context = caption words (W) ----
        re_i2t = _focal_attn(jnp.swapaxes(raw, -1, -2))  # (C,I,R,W)
        num2 = jnp.einsum("cirw,cirw->cir", re_i2t, raw)
        qn2 = jnp.einsum("cirw,cwv,cirv->cir", re_i2t, g_cap, re_i2t)
        qn2 = jnp.maximum(jnp.sqrt(qn2), EPS)
        i2t = (num2 / (img_norm[None, :, :] * qn2)).mean(axis=-1)  # (C,I)

        return (t2i + i2t).T  # (I, C_loc)

    pfn = jax.pmap(_shard_fn, in_axes=(None, 0))
    _compiled["pfn"] = pfn
    return pfn


def kernel(images, captions, cap_lens):
    import jax.numpy as jnp

    pfn = _get_pfn()
    images = jnp.asarray(np.asarray(images, dtype=np.float32))
    cap_sh = np.asarray(captions, dtype=np.float32).reshape(
        N_CORES, N_CAPTION // N_CORES, N_WORD, D
    )
    out = pfn(images, jnp.asarray(cap_sh))  # (8, I, C/8)
    out = np.asarray(out)
    return np.concatenate([out[k] for k in range(N_CORES)], axis=1).astype(np.float32)


# revision 3
# speedup vs baseline: 1.0115x; 1.0115x over previous
"""BFAN sparse-attention similarity kernel, sharded across 8 trn2 NeuronCores.

Strategy (per sharding hint): shard the caption axis across the 8 cores.
Each core computes a (n_image=128, n_caption/8=16) block of the similarity
matrix with images replicated; the host concatenates the column blocks.

Shapes (hardcoded per the problem spec):
  images:   (128, 36, 1024) f32
  captions: (128, 32, 1024) f32
  cap_lens: (128,) int32  (all == 32; unused by the math)
Output: (128, 128) f32  -> out[i, c]
"""

import numpy as np

N_IMAGE, N_REGIONS = 128, 36
N_CAPTION, N_WORD = 128, 32
D = 1024
LAMBDA_SOFTMAX = 20.0
EPS = 1e-8
LEAKY_SLOPE = 0.1
N_CORES = 8

_compiled = {}


def _get_pfn():
    if "pfn" in _compiled:
        return _compiled["pfn"]

    import jax
    import jax.numpy as jnp

    def _l2norm(x, axis):
        return x / (jnp.linalg.norm(x, axis=axis, keepdims=True) + EPS)

    def _focal_attn(raw):
        a = jax.nn.leaky_relu(raw, LEAKY_SLOPE)
        a = _l2norm(a, axis=-1)
        a = jnp.swapaxes(a, -1, -2)
        a = jax.nn.softmax(a * LAMBDA_SOFTMAX, axis=-1)
        S = a.shape[-1]
        funcF = a * S - jnp.sum(a, axis=-1, keepdims=True)
        fattn = jnp.where(funcF > 0, 1.0, 0.0).astype(a.dtype)
        tmp = fattn * a
        return tmp / jnp.sum(tmp, axis=-1, keepdims=True)

    def _shard_fn(img_shard, captions):
        # img_shard: (I/8, R, d) -> all-gather on-chip to avoid replicating
        # the full image tensor over the host->device link 8 times.
        images = jax.lax.all_gather(img_shard, "cores", axis=0).reshape(
            N_IMAGE, N_REGIONS, D
        )
        # images: (I, R, d); captions: (C_loc, W, d)
        # Use the Gram-matrix reformulation to avoid materializing the
        # (C, I, W, d) weighted-context tensors: for attention weights re and
        # raw similarity matrix raw,
        #   cos(query_q, wctx_q) = sum_s re[q,s] raw[s,q]
        #       / (||query_q|| * sqrt(re[q,:] @ G @ re[q,:]^T))
        # where G is the Gram matrix of the context set.
        raw = jnp.einsum("ird,cwd->cirw", images, captions)  # (C,I,R,W)
        g_img = jnp.einsum("ird,isd->irs", images, images)  # (I,R,R)
        g_cap = jnp.einsum("cwd,cvd->cwv", captions, captions)  # (C,W,W)
        img_norm = jnp.maximum(jnp.linalg.norm(images, axis=-1), EPS)  # (I,R)
        cap_norm = jnp.maximum(jnp.linalg.norm(captions, axis=-1), EPS)  # (C,W)

        # ---- t2i: query = caption words (W), context = image regions (R) ----
        re_t2i = _focal_attn(raw)  # (C,I,W,R)
        num = jnp.einsum("ciwr,cirw->ciw", re_t2i, raw)
        qn = jnp.einsum("ciwr,irs,ciws->ciw", re_t2i, g_img, re_t2i)
        qn = jnp.maximum(jnp.sqrt(qn), EPS)
        t2i = (num / (cap_norm[:, None, :] * qn)).mean(axis=-1)  # (C,I)

        # ---- i2t: query = image regions (R), context = caption words (W) ----
        re_i2t = _focal_attn(jnp.swapaxes(raw, -1, -2))  # (C,I,R,W)
        num2 = jnp.einsum("cirw,cirw->cir", re_i2t, raw)
        qn2 = jnp.einsum("cirw,cwv,cirv->cir", re_i2t, g_cap, re_i2t)
        qn2 = jnp.maximum(jnp.sqrt(qn2), EPS)
        i2t = (num2 / (img_norm[None, :, :] * qn2)).mean(axis=-1)  # (C,I)

        return (t2i + i2t).T  # (I, C_loc)

    pfn = jax.pmap(_shard_fn, axis_name="cores", in_axes=(0, 0))
    _compiled["pfn"] = pfn
    return pfn


def kernel(images, captions, cap_lens):
    import jax.numpy as jnp

    pfn = _get_pfn()
    img_sh = np.asarray(images, dtype=np.float32).reshape(
        N_CORES, N_IMAGE // N_CORES, N_REGIONS, D
    )
    cap_sh = np.asarray(captions, dtype=np.float32).reshape(
        N_CORES, N_CAPTION // N_CORES, N_WORD, D
    )
    out = pfn(jnp.asarray(img_sh), jnp.asarray(cap_sh))  # (8, I, C/8)
    out = np.asarray(out)
    return np.concatenate([out[k] for k in range(N_CORES)], axis=1).astype(np.float32)


# revision 4
# speedup vs baseline: 10.2329x; 10.1165x over previous
"""BFAN sparse-attention similarity kernel, sharded across 8 trn2 NeuronCores.

Strategy (per sharding hint): shard the caption axis across the 8 cores.
Each core computes a (n_image=128, n_caption/8=16) block of the similarity
matrix with images replicated; the host concatenates the column blocks.

Shapes (hardcoded per the problem spec):
  images:   (128, 36, 1024) f32
  captions: (128, 32, 1024) f32
  cap_lens: (128,) int32  (all == 32; unused by the math)
Output: (128, 128) f32  -> out[i, c]
"""

import numpy as np

N_IMAGE, N_REGIONS = 128, 36
N_CAPTION, N_WORD = 128, 32
D = 1024
LAMBDA_SOFTMAX = 20.0
EPS = 1e-8
LEAKY_SLOPE = 0.1
N_CORES = 8

_compiled = {}


def _get_pfn():
    if "pfn" in _compiled:
        return _compiled["pfn"]

    import jax
    import jax.numpy as jnp

    def _l2norm(x, axis):
        return x / (jnp.linalg.norm(x, axis=axis, keepdims=True) + EPS)

    def _focal_attn(raw):
        a = jax.nn.leaky_relu(raw, LEAKY_SLOPE)
        a = _l2norm(a, axis=-1)
        a = jnp.swapaxes(a, -1, -2)
        a = jax.nn.softmax(a * LAMBDA_SOFTMAX, axis=-1)
        S = a.shape[-1]
        funcF = a * S - jnp.sum(a, axis=-1, keepdims=True)
        fattn = jnp.where(funcF > 0, 1.0, 0.0).astype(a.dtype)
        tmp = fattn * a
        return tmp / jnp.sum(tmp, axis=-1, keepdims=True)

    def _shard_fn(img_shard, captions):
        # img_shard: (I/8, R, d) -> all-gather on-chip to avoid replicating
        # the full image tensor over the host->device link 8 times.
        images = jax.lax.all_gather(img_shard, "cores", axis=0).reshape(
            N_IMAGE, N_REGIONS, D
        )
        # images: (I, R, d); captions: (C_loc, W, d)
        # Use the Gram-matrix reformulation to avoid materializing the
        # (C, I, W, d) weighted-context tensors: for attention weights re and
        # raw similarity matrix raw,
        #   cos(query_q, wctx_q) = sum_s re[q,s] raw[s,q]
        #       / (||query_q|| * sqrt(re[q,:] @ G @ re[q,:]^T))
        # where G is the Gram matrix of the context set.
        raw = jnp.einsum(
            "ird,cwd->cirw",
            images.astype(jnp.bfloat16),
            captions.astype(jnp.bfloat16),
            preferred_element_type=jnp.float32,
        )  # (C,I,R,W)
        g_img = jnp.einsum("ird,isd->irs", images, images)  # (I,R,R)
        g_cap = jnp.einsum("cwd,cvd->cwv", captions, captions)  # (C,W,W)
        img_norm = jnp.maximum(jnp.linalg.norm(images, axis=-1), EPS)  # (I,R)
        cap_norm = jnp.maximum(jnp.linalg.norm(captions, axis=-1), EPS)  # (C,W)

        # ---- t2i: query = caption words (W), context = image regions (R) ----
        re_t2i = _focal_attn(raw)  # (C,I,W,R)
        num = jnp.einsum("ciwr,cirw->ciw", re_t2i, raw)
        qn = jnp.einsum("ciwr,irs,ciws->ciw", re_t2i, g_img, re_t2i)
        qn = jnp.maximum(jnp.sqrt(qn), EPS)
        t2i = (num / (cap_norm[:, None, :] * qn)).mean(axis=-1)  # (C,I)

        # ---- i2t: query = image regions (R), context = caption words (W) ----
        re_i2t = _focal_attn(jnp.swapaxes(raw, -1, -2))  # (C,I,R,W)
        num2 = jnp.einsum("cirw,cirw->cir", re_i2t, raw)
        qn2 = jnp.einsum("cirw,cwv,cirv->cir", re_i2t, g_cap, re_i2t)
        qn2 = jnp.maximum(jnp.sqrt(qn2), EPS)
        i2t = (num2 / (img_norm[None, :, :] * qn2)).mean(axis=-1)  # (C,I)

        return (t2i + i2t).T  # (I, C_loc)

    pfn = jax.pmap(_shard_fn, axis_name="cores", in_axes=(0, 0))
    _compiled["pfn"] = pfn
    return pfn


def kernel(images, captions, cap_lens):
    import jax.numpy as jnp

    pfn = _get_pfn()
    img_sh = np.asarray(images, dtype=np.float32).reshape(
        N_CORES, N_IMAGE // N_CORES, N_REGIONS, D
    )
    cap_sh = np.asarray(captions, dtype=np.float32).reshape(
        N_CORES, N_CAPTION // N_CORES, N_WORD, D
    )
    out = pfn(jnp.asarray(img_sh), jnp.asarray(cap_sh))  # (8, I, C/8)
    out = np.asarray(out)
    return np.concatenate([out[k] for k in range(N_CORES)], axis=1).astype(np.float32)
